# revision 1
# baseline (speedup 1.0000x reference)
"""Trainium2 Bass kernel for nn_DiffNet (gnn_message_passing).

The reference's per-element "edge MLP" over the meta stack
(vi, W, vj) -> two 1x1 convs -> weighted sum over the input dim is
linear in its 3 channels, so it collapses algebraically.  With
g = conv1_w.T @ conv2_w[0]  (3 scalars), hb = conv1_b@conv2_w[0]+conv2_b[0],
z = vi @ W.T (no bias), s1[b] = sum_i vi[b,i], s2[b] = sum_i vi[b,i]^2:

    out[b,o] = relu(z+b)[b,o] * (1 + scale*g2*s1[b])
             + scale*(g0*s2[b] + g1*z[b,o] + hb*s1[b])

so the whole network is 3 small matmuls + elementwise, and the problem
is memory-bound on the fc weights (3.5 MB fp32).

Distribution (8 cores, no collectives): fc1/fc2 replicated (any
zero-communication scheme must read them on every core since every
output depends on all of them), fc3 sharded over its output dim
(32 cols/core); full batch B=32 on every core; host concatenates the
8 [32,32] output shards.

On-core layout: activations live transposed [feature(partitions), batch]
in 128-row chunks; weights are passed pre-transposed [in, out] so matmuls
need no on-chip weight transpose.  Matmuls put the (tiny) activation
tile stationary and stream the weight chunk [128, 512] as the moving
operand in float32r (1 cycle/row at N>=512 vs 4 for plain fp32); all
tensors on the matmul dataflow are declared float32r so their producers
satisfy the walrus fp32r-rounding rule.  The z output lands
[batch, out]; a cheap PE transpose brings each 128-col chunk back to
[out, batch] where relu-bias (per-partition), the k1*z term and the
per-batch alpha/beta scalars (broadcast across partitions via a rank-1
ones matmul) are applied with a few wide DVE ops.
"""

import sys

if "/opt/trn_rl_repo" not in sys.path:
    sys.path.insert(0, "/opt/trn_rl_repo")

import numpy as np


def _install_ntff_hook_shim():
    """This image's antenv lacks ``axon_hooks``; bass_utils hard-imports it
    when tracing under axon.  Provide the module and register the ctypes
    NTFF hook from trn_agent_boot so ``trace=True`` yields exec_time_ns."""
    import types

    if "antenv.axon_hooks" in sys.modules:
        return
    try:
        import antenv

        mod = types.ModuleType("antenv.axon_hooks")
        _h = [None]
        mod.set_axon_ntff_profile_hook = lambda hook: _h.__setitem__(0, hook)
        mod.get_axon_ntff_profile_hook = lambda: _h[0]
        sys.modules["antenv.axon_hooks"] = mod
        antenv.axon_hooks = mod
        from trn_agent_boot.trn_boot import _ntff_profile_via_ctypes

        mod.set_axon_ntff_profile_hook(
            _ntff_profile_via_ctypes("/opt/axon/libaxon_pjrt.so")
        )
    except Exception:
        pass


_install_ntff_hook_shim()

N_CORES = 8
B = 32
I1, O1, O2, O3 = 1024, 512, 512, 256
O3L = O3 // N_CORES  # fc3 output cols per core
RATE = 0.1

_CACHE = {}
LAST_RESULTS = None  # BassKernelResults of the most recent run (for test.py)


def _build(k0, k1, k2, kb):
    import concourse.bacc as bacc
    import concourse.mybir as mybir
    import concourse.tile as tile
    import concourse.bass as bass

    f32 = mybir.dt.float32
    f32r = mybir.dt.float32r
    AF = mybir.ActivationFunctionType
    ALU = mybir.AluOpType

    from concourse.tile_rust import add_dep_helper

    nc = bacc.Bacc(
        "TRN2", target_bir_lowering=False, debug=False, num_devices=N_CORES
    )

    f16 = mybir.dt.float16
    # x is f32r (the DMA *rounds* f32r payloads — only matmul operands may
    # travel that way); everything else rides a plain-f32 misc tensor.
    # misc cols: [b12: 0..8) [b3: 8] [eye: 9..41) [onesK: 41]
    # [Kalpha f16 [3,128] packed in f32: 42..106) [Kbeta f16: 106..170)
    XW = 8 * B
    MW = 42 + 128
    xm = nc.declare_dram_parameter("xm", [128, XW], f32r, isOutput=False)
    misc = nc.declare_dram_parameter("misc", [128, MW], f32, isOutput=False)
    w1 = nc.declare_dram_parameter("w1t", [128, 8 * O1], f32r, isOutput=False)
    w2 = nc.declare_dram_parameter("w2t", [128, 4 * O2], f32r, isOutput=False)
    w3 = nc.declare_dram_parameter("w3t", [128, 4 * O3L], f32r, isOutput=False)
    out_d = nc.declare_dram_parameter("out", [O3L, B], f32, isOutput=True)

    with tile.TileContext(nc) as tc:
        with (
            tc.tile_pool(name="wts", bufs=1) as wp,
            tc.tile_pool(name="act", bufs=1) as ap,
            tc.tile_pool(name="ps", bufs=1, space=bass.MemorySpace.PSUM) as pp,
        ):
            tw1 = wp.tile([128, 8 * O1], f32r, tag="w1")
            tw2 = wp.tile([128, 4 * O2], f32r, tag="w2")
            tw3 = wp.tile([128, 4 * O3L], f32r, tag="w3")
            txm = wp.tile([128, XW], f32r, tag="xm")
            tx = txm[:]  # f32r activations for layer 1
            tmisc = wp.tile([128, MW], f32, tag="misc")
            tb12 = tmisc[:, 0:8]
            tb3 = tmisc[0:O3L, 8:9]
            teye = tmisc[0:B, 9:41]
            t1k = tmisc[:, 41:42]  # f32 ones col (K-dir sums)
            # coefficient matrices for the alpha/beta broadcast matmuls:
            # alpha/beta(p, b) = K.T @ s_sb(:, b), sources on rows 0/32/64
            tka16 = tmisc[0:96, 42:106].bitcast(f16)  # [96,128] f16
            tkb16 = tmisc[0:96, 106:170].bitcast(f16)  # [96,128] f16

            # -- DMAs: one HWDGE ring, in need-order, few enough that each
            # gets its own completion-sem lane.  fc1 in thirds so its
            # z-matmuls start as the stream lands.
            nc.sync.dma_start(tmisc[:], misc[:])
            nc.sync.dma_start(txm[:], xm[:])
            for lo, hi in ((0, 3), (3, 6), (6, 8)):
                nc.sync.dma_start(
                    tw1[:, lo * O1 : hi * O1], w1[:, lo * O1 : hi * O1]
                )
            nc.sync.dma_start(tw2[:], w2[:])
            nc.sync.dma_start(tw3[:], w3[:])

            def ordered(dependent, dependency, why):
                if dependent is not None and dependency is not None:
                    add_dep_helper(
                        dependent.ins, dependency.ins, sync=False, reason=why
                    )

            def stats_ab(a_tile, n_c, tag, after_mm=None):
                """a_tile [128, n_c*B] float32r; -> (ab_sb [128, 2*B], bcast).
                ab rows all equal; cols 0:B = alpha(b), B:2B = beta(b).
                Everything on the f32r single-pass path: squares come from a
                DVE multiply writing f32r (the walrus fp32r-producer rule
                allows DVE outputs), so both column-sum chains are f32r."""
                asq = ap.tile([128, n_c * B], f32r, tag=tag + "sq")
                af = a_tile.bitcast(f32)
                nc.vector.tensor_tensor(asq[:], af, af, ALU.mult)
                s1_ps = pp.tile([1, B], f32, tag="s1")
                s2_ps = pp.tile([1, B], f32, tag="s2")
                mm1 = None
                for c in range(n_c):
                    mm = nc.tensor.matmul(
                        s1_ps[:],
                        t1k,
                        af[:, c * B : (c + 1) * B],
                        start=(c == 0),
                        stop=(c == n_c - 1),
                    )
                    mm1 = mm1 or mm
                asqf = asq[:].bitcast(f32)
                for c in range(n_c):
                    nc.tensor.matmul(
                        s2_ps[:],
                        t1k,
                        asqf[:, c * B : (c + 1) * B],
                        start=(c == 0),
                        stop=(c == n_c - 1),
                    )
                ordered(mm1, after_mm, "stats after this layer's z matmuls")
                # engine writes must start at partition 0/32/64 -> spread
                # (s1, s2, 1) over those rows; memset first so junk
                # partitions are finite (their K coefficients are 0) and
                # row 64 is the ones row
                s_sb = ap.tile([96, B], f16, tag=tag + "row")
                nc.vector.memset(s_sb[:], 1.0)
                nc.scalar.copy(s_sb[0:1, :], s1_ps[:])
                nc.scalar.copy(s_sb[32:33, :], s2_ps[:])
                ab_ps = pp.tile([128, 2 * B], f32, tag="ab")
                nc.tensor.matmul(
                    ab_ps[:, 0:B], tka16, s_sb[:], start=True, stop=True
                )
                bcast = nc.tensor.matmul(
                    ab_ps[:, B : 2 * B], tkb16, s_sb[:], start=True, stop=True
                )
                ab_sb = ap.tile([128, 2 * B], f32, tag=tag + "sb")
                nc.scalar.copy(ab_sb[:], ab_ps[:])
                return ab_sb, bcast

            def z_mms(a_tile, w_tile, n_ic, ow, after=None):
                """z_ps [B, ow] = a.T @ w, accumulated over n_ic chunks."""
                z_ps = pp.tile([B, ow], f32, tag="z")
                last = None
                for ic in range(n_ic):
                    mm = nc.tensor.matmul(
                        z_ps[:],
                        a_tile[:, ic * B : (ic + 1) * B],
                        w_tile[:, ic * ow : (ic + 1) * ow],
                        start=(ic == 0),
                        stop=(ic == n_ic - 1),
                    )
                    if ic == 0:
                        ordered(mm, after, "z matmuls after stats bcast")
                    last = mm
                return z_ps, last

            def tail(z_ps, n_oc, ow, bias_col, ab_sb, out_view, li, after=None):
                """transpose z back to [out, batch]; relu+bias on DVE;
                combine with alpha/beta; writes out_view [np_out, n_oc*B]."""
                np_out = min(ow, 128)
                z_sb = ap.tile([B, ow], f32, tag=f"zsb{li}")
                nc.scalar.copy(z_sb[:], z_ps[:])
                vjt = ap.tile([np_out, n_oc * B], f32, tag=f"vj{li}")
                t_sb = ap.tile([np_out, n_oc * B], f32, tag=f"t{li}")
                alpha = ab_sb[0:np_out, 0:B]
                beta = ab_sb[0:np_out, B : 2 * B]
                for oc in range(n_oc):
                    bsl = slice(oc * B, (oc + 1) * B)
                    # separate PSUM tile per oc: PE transpose-writes and
                    # DVE/ACT reads of different chunks must not serialize
                    # on Tile's per-tile bank tracking
                    zt_ps = pp.tile([np_out, B], f32, tag=f"zt{oc}")
                    tr = nc.tensor.transpose(
                        zt_ps[:],
                        z_sb[:, oc * 128 : oc * 128 + np_out],
                        teye,
                    )
                    if oc == 0:
                        ordered(tr, after, "transposes after stats bcast")
                    # relu(z + bias): alternate ACT / DVE so neither engine
                    # paces the per-oc pipeline
                    if oc % 2 == 0:
                        nc.scalar.activation(
                            vjt[:, bsl], zt_ps[:], AF.Relu,
                            bias=bias_col(oc), scale=1.0,
                        )
                    else:
                        nc.vector.tensor_scalar(
                            vjt[:, bsl], zt_ps[:], bias_col(oc), 0.0,
                            ALU.add, ALU.max,
                        )
                    # t = k1*z + beta ; out = vj*alpha + t  (per-oc so the
                    # next layer's matmul ic can start as soon as its input
                    # chunk exists)
                    nc.vector.scalar_tensor_tensor(
                        t_sb[:, bsl], zt_ps[:], k1, beta, ALU.mult, ALU.add
                    )
                    nc.vector.tensor_tensor(
                        vjt[:, bsl], vjt[:, bsl], alpha, ALU.mult
                    )
                    nc.vector.tensor_tensor(
                        out_view[:, bsl], vjt[:, bsl], t_sb[:, bsl], ALU.add
                    )

            # ---- forward chain: stats1 fills the PE while fc1 streams in;
            # later layers run stats between their z matmuls and transposes.
            ab1, bc1 = stats_ab(tx, 8, "ab1")
            z1, z1l = z_mms(tx, tw1, 8, O1, after=bc1)
            a2 = ap.tile([128, 4 * B], f32r, tag="a2")
            tail(z1, 4, O1, lambda oc: tb12[:, oc : oc + 1], ab1, a2[:], 1)

            z2, z2l = z_mms(a2[:], tw2, 4, O2)
            ab2, bc2 = stats_ab(a2[:], 4, "ab2", after_mm=z2l)
            a3 = ap.tile([128, 4 * B], f32r, tag="a3")
            tail(z2, 4, O2, lambda oc: tb12[:, 4 + oc : 5 + oc], ab2, a3[:], 2,
                 after=bc2)

            z3, z3l = z_mms(a3[:], tw3, 4, O3L)
            ab3, bc3 = stats_ab(a3[:], 4, "ab3", after_mm=z3l)
            out_sb = ap.tile([O3L, B], f32, tag="o3")
            tail(z3, 1, O3L, lambda oc: tb3, ab3, out_sb[:], 3, after=bc3)

            nc.sync.dma_start(out_d[:], out_sb[:])

    nc.compile()
    return nc


def kernel(**inputs):
    from concourse.bass_utils import run_bass_kernel_spmd

    x = np.ascontiguousarray(np.asarray(inputs["x"], dtype=np.float32))
    fc1_w = np.asarray(inputs["fc1_w"], dtype=np.float32)
    fc1_b = np.asarray(inputs["fc1_b"], dtype=np.float32)
    fc2_w = np.asarray(inputs["fc2_w"], dtype=np.float32)
    fc2_b = np.asarray(inputs["fc2_b"], dtype=np.float32)
    fc3_w = np.asarray(inputs["fc3_w"], dtype=np.float32)
    fc3_b = np.asarray(inputs["fc3_b"], dtype=np.float32)
    c1w = np.asarray(inputs["conv1_w"], dtype=np.float32)
    c1b = np.asarray(inputs["conv1_b"], dtype=np.float32)
    c2w = np.asarray(inputs["conv2_w"], dtype=np.float32)
    c2b = np.asarray(inputs["conv2_b"], dtype=np.float32)
    bn = float(np.asarray(inputs["batch_num"]).astype(np.float64))

    scale = np.float32(RATE) / np.float32(bn)
    g = (c1w.T @ c2w[0]).astype(np.float32)  # [3]
    hb = np.float32(c1b @ c2w[0] + c2b[0])
    k0 = float(scale * g[0])
    k1 = float(scale * g[1])
    k2 = float(scale * g[2])
    kb = float(scale * hb)

    key = (k0, k1, k2, kb)
    if key not in _CACHE:
        _CACHE[key] = _build(*key)
    nc = _CACHE[key]

    def pack(m, n_c, width):  # [n_c*128, width] -> [128, n_c*width]
        return np.ascontiguousarray(
            m.reshape(n_c, 128, width).transpose(1, 0, 2).reshape(128, n_c * width)
        )

    w1_h = pack(fc1_w.T, 8, O1)
    w2_h = pack(fc2_w.T, 4, O2)
    xm_h = pack(x.T, 8, B)
    # misc layout must match _build: b12 | b3 | eye | onesK | Kalpha | Kbeta
    MW = 42 + 128
    misc_h = np.zeros((128, MW), dtype=np.float32)
    misc_h[:, 0:4] = fc1_b.reshape(4, 128).T
    misc_h[:, 4:8] = fc2_b.reshape(4, 128).T
    misc_h[0:B, 9:41] = np.eye(B, dtype=np.float32)
    misc_h[:, 41] = 1.0  # ones col (K-dir sums)
    ka_m = np.zeros((96, 128), np.float16)
    ka_m[0, :] = k2
    ka_m[64, :] = 1.0  # alpha = k2*s1 + 1
    kb_m = np.zeros((96, 128), np.float16)
    kb_m[0, :] = kb
    kb_m[32, :] = k0  # beta = kb*s1 + k0*s2
    misc_h[0:96, 42:106] = ka_m.view(np.float32)
    misc_h[0:96, 106:170] = kb_m.view(np.float32)

    in_maps = []
    for c in range(N_CORES):
        w3_h = pack(fc3_w[c * O3L : (c + 1) * O3L].T, 4, O3L)
        m_h = misc_h.copy()
        m_h[0:O3L, 8] = fc3_b[c * O3L : (c + 1) * O3L]
        in_maps.append(
            dict(xm=xm_h, misc=m_h, w1t=w1_h, w2t=w2_h, w3t=w3_h)
        )

    res = run_bass_kernel_spmd(nc, in_maps, list(range(N_CORES)))
    global LAST_RESULTS
    LAST_RESULTS = res
    return np.ascontiguousarray(
        np.concatenate([res.results[c]["out"].T for c in range(N_CORES)], axis=1)
    ).astype(np.float32)


if __name__ == "__main__":
    rng = np.random.default_rng(0)

    def lin(fo, fi):
        bound = 1.0 / np.sqrt(fi)
        return (
            rng.uniform(-bound, bound, (fo, fi)).astype(np.float32),
            rng.uniform(-bound, bound, (fo,)).astype(np.float32),
        )

    fc1_w, fc1_b = lin(512, 1024)
    fc2_w, fc2_b = lin(512, 512)
    fc3_w, fc3_b = lin(256, 512)
    c1w, c1b = lin(8, 3)
    c2w, c2b = lin(1, 8)
    ins = dict(
        x=rng.standard_normal((32, 1024)).astype(np.float32),
        fc1_w=fc1_w, fc1_b=fc1_b, fc2_w=fc2_w, fc2_b=fc2_b,
        fc3_w=fc3_w, fc3_b=fc3_b,
        conv1_w=c1w, conv1_b=c1b, conv2_w=c2w, conv2_b=c2b,
        batch_num=10,
    )
    out = kernel(**ins)
    print("kernel out", out.shape, out.dtype, float(np.abs(out).max()))



# revision 6
# speedup vs baseline: 1.2834x; 1.2834x over previous
"""Trainium2 Bass kernel for nn_DiffNet (gnn_message_passing).

The reference's per-element "edge MLP" over the meta stack (vi, W, vj)
collapses algebraically: with g = conv1_w.T @ conv2_w[0],
hb = conv1_b@conv2_w[0]+conv2_b[0], z = vi @ W.T (no bias),
s1[b] = sum_i vi[b,i], s2[b] = sum_i vi[b,i]^2:

    out = relu(z+b)*(1 + k2*s1) + k1*z + (k0*s2 + kb*s1)

so the network is 3 matmuls + elementwise.  Distribution: fc1/fc2
replicated (zero-communication), fc3 sharded over its output dim
(32 cols/core); host concatenates the 8 [32,32] shards.

v2 design (vs the fp32 v1 at ~33us):
 - everything on the PE dataflow is fp16 (rel err ~1.5e-3, gate 2e-2):
   halves HBM traffic to ~1.7MB/core and runs matmuls at 1 col/cycle.
 - W-stationary matmuls: stationary = W.T tile [128 in, 128 out], moving
   = activations [128, 32]; z lands [out_feature, batch] in PSUM, so the
   per-layer tail (relu-bias per partition, alpha/beta per batch-column)
   needs NO transposes and no PSUM->SBUF z copies.
 - broadcast-stats: a ones[128,128] stationary matmul broadcasts
   s1 = sum_i a and s2 = sum_i a^2 to all 128 partitions directly, so
   alpha/beta are two small DVE ops, no row-placement/second matmul.
 - w1 streams in two 512KB DMA slices (mg-major layout) so the first
   two output col-groups' z chains + tail overlap the second slice.
 - ~2.2us of warmup matmuls run during the DMA-only window to lift the
   PE HAM clock gate from 1.2 to 2.4 GHz before the real chains start.
"""

import sys

if "/opt/trn_rl_repo" not in sys.path:
    sys.path.insert(0, "/opt/trn_rl_repo")

import numpy as np


def _install_ntff_hook_shim():
    """This image's antenv lacks ``axon_hooks``; bass_utils hard-imports it
    when tracing under axon.  Provide the module and register the ctypes
    NTFF hook from trn_agent_boot so ``trace=True`` yields exec_time_ns."""
    import types

    if "antenv.axon_hooks" in sys.modules:
        return
    try:
        import antenv

        mod = types.ModuleType("antenv.axon_hooks")
        _h = [None]
        mod.set_axon_ntff_profile_hook = lambda hook: _h.__setitem__(0, hook)
        mod.get_axon_ntff_profile_hook = lambda: _h[0]
        sys.modules["antenv.axon_hooks"] = mod
        antenv.axon_hooks = mod
        from trn_agent_boot.trn_boot import _ntff_profile_via_ctypes

        mod.set_axon_ntff_profile_hook(
            _ntff_profile_via_ctypes("/opt/axon/libaxon_pjrt.so")
        )
    except Exception:
        pass


_install_ntff_hook_shim()

N_CORES = 8
B = 32
I1, O1, O2, O3 = 1024, 512, 512, 256
O3L = O3 // N_CORES  # fc3 output cols per core
RATE = 0.1
N_WARMUP_MM = 18

_CACHE = {}
LAST_RESULTS = None  # BassKernelResults of the most recent run (for test.py)


def _build(k0, k1, k2, kb):
    import concourse.bacc as bacc
    import concourse.mybir as mybir
    import concourse.tile as tile
    import concourse.bass as bass

    f32 = mybir.dt.float32
    f16 = mybir.dt.float16
    AF = mybir.ActivationFunctionType
    ALU = mybir.AluOpType

    nc = bacc.Bacc(
        "TRN2", target_bir_lowering=False, debug=False, num_devices=N_CORES
    )

    # DRAM layouts (all packed on host):
    # xm:  x.T in 8 chunks [128, 32] fp16 -> [128, 256]
    # w1:  W1.T mg-major: for mg(4): for ic(8): [128,128] -> [128, 4096]
    # w2:  W2.T mg-major: for mg(4): for ic(4): [128,128] -> [128, 2048]
    # w3:  W3shard.T:     for ic(4): [128, 32]            -> [128, 128]
    # misc f32: cols 0:4 = fc1_b by mg, 4:8 = fc2_b by mg, 8 = fc3_b shard
    xm = nc.declare_dram_parameter("xm", [128, 8 * B], f16, isOutput=False)
    misc = nc.declare_dram_parameter("misc", [128, 12], f32, isOutput=False)
    w1 = nc.declare_dram_parameter("w1m", [128, 4096], f16, isOutput=False)
    w2 = nc.declare_dram_parameter("w2m", [128, 2048], f16, isOutput=False)
    w3 = nc.declare_dram_parameter("w3m", [128, 4 * O3L], f16, isOutput=False)
    out_d = nc.declare_dram_parameter("out", [O3L, B], f32, isOutput=True)

    with tile.TileContext(nc) as tc:
        with (
            tc.tile_pool(name="wts", bufs=1) as wp,
            tc.tile_pool(name="act", bufs=1) as ap,
            tc.tile_pool(name="ps", bufs=1, space=bass.MemorySpace.PSUM) as pp,
        ):
            tx = wp.tile([128, 8 * B], f16, tag="xm")
            tmisc = wp.tile([128, 12], f32, tag="misc")
            tw1 = wp.tile([128, 4096], f16, tag="w1")
            tw2 = wp.tile([128, 2048], f16, tag="w2")
            tw3 = wp.tile([128, 4 * O3L], f16, tag="w3")
            tones = wp.tile([128, 128], f16, tag="ones")
            nc.vector.memset(tones[:], 1.0)

            # DMAs in strict need-order on the sync HWDGE queue; tiny misc
            # rides the scalar HWDGE queue in parallel.
            nc.scalar.dma_start(tmisc[:], misc[:])
            nc.sync.dma_start(tx[:], xm[:])
            nc.sync.dma_start(tw1[:, 0:2048], w1[:, 0:2048])
            nc.sync.dma_start(tw1[:, 2048:4096], w1[:, 2048:4096])
            nc.sync.dma_start(tw2[:], w2[:])
            nc.sync.dma_start(tw3[:], w3[:])

            # PE warmup: keep the array busy from t~0.5us so the HAM clock
            # gate reaches 8/8 (2.4 GHz) before the real chains start.
            # (shares the "za" PSUM bank; the z1 chain's start=True resets it)
            junk = pp.tile([128, 64], f32, tag="za")
            for _ in range(N_WARMUP_MM):
                nc.tensor.matmul(
                    junk[:], tones[:], tones[:, 0:64], start=True, stop=True
                )

            def stats(a_sb, asq_sb, n_c):
                """s_ps [128, 64]: cols 0:32 = s1 bcast, 32:64 = s2 bcast."""
                s_ps = pp.tile([128, 64], f32, tag="s")
                for c in range(n_c):
                    nc.tensor.matmul(
                        s_ps[:, 0:32],
                        tones[:],
                        a_sb[:, c * B : (c + 1) * B],
                        start=(c == 0),
                        stop=(c == n_c - 1),
                    )
                for c in range(n_c):
                    nc.tensor.matmul(
                        s_ps[:, 32:64],
                        tones[:],
                        asq_sb[:, c * B : (c + 1) * B],
                        start=(c == 0),
                        stop=(c == n_c - 1),
                    )
                return s_ps

            def ab_from(s_ps, tag):
                """alpha (f16) and beta (f32) [128, 64], duplicated halves so
                the 64-wide tail ops can use them directly."""
                alpha = ap.tile([128, 64], f16, tag="al" + tag)
                beta = ap.tile([128, 64], f32, tag="be" + tag)
                tmpb = ap.tile([128, 32], f32, tag="tb" + tag)
                nc.vector.tensor_scalar(
                    tmpb[:], s_ps[:, 32:64], k0, 0.0, ALU.mult, ALU.add
                )
                for h in range(2):
                    hs = slice(h * 32, (h + 1) * 32)
                    nc.vector.tensor_scalar(
                        alpha[:, hs], s_ps[:, 0:32], k2, 1.0, ALU.mult, ALU.add
                    )
                    nc.vector.scalar_tensor_tensor(
                        beta[:, hs], s_ps[:, 0:32], kb, tmpb[:], ALU.mult, ALU.add
                    )
                return alpha, beta

            def z_half(w_t, a_sb, n_c, half):
                """One [128, 64] PSUM tile holding two mg chains (mg=2h, 2h+1)."""
                z_ps = pp.tile([128, 64], f32, tag="za" if half == 0 else "zb")
                for m in range(2):
                    mg = 2 * half + m
                    for ic in range(n_c):
                        nc.tensor.matmul(
                            z_ps[:, m * B : (m + 1) * B],
                            w_t[:, (mg * n_c + ic) * 128 : (mg * n_c + ic + 1) * 128],
                            a_sb[:, ic * B : (ic + 1) * B],
                            start=(ic == 0),
                            stop=(ic == n_c - 1),
                        )
                return z_ps

            def tail_half(z_ps, alpha, beta, bcol, a_view, asq_view, li, h):
                """a_next[:, 64h:64h+64] = relu(z+b)*alpha + k1*z + beta."""
                vj = ap.tile([128, 64], f16, tag=f"vj{li}{h}")
                t1 = ap.tile([128, 64], f32, tag=f"t{li}{h}")
                for m in range(2):
                    ms = slice(m * B, (m + 1) * B)
                    nc.scalar.activation(
                        vj[:, ms], z_ps[:, ms], AF.Relu,
                        bias=tmisc[:, bcol + m : bcol + m + 1], scale=1.0,
                    )
                nc.vector.scalar_tensor_tensor(
                    t1[:], z_ps[:], k1, beta[:], ALU.mult, ALU.add
                )
                nc.vector.tensor_tensor(vj[:], vj[:], alpha[:], ALU.mult)
                nc.vector.tensor_tensor(a_view, vj[:], t1[:], ALU.add)
                nc.vector.tensor_tensor(asq_view, a_view, a_view, ALU.mult)

            # ---- layer 1 ----
            xsq = ap.tile([128, 8 * B], f16, tag="xsq")
            nc.vector.tensor_tensor(xsq[:], tx[:], tx[:], ALU.mult)
            s1p = stats(tx[:], xsq[:], 8)
            al1, be1 = ab_from(s1p, "1")
            a2 = ap.tile([128, 128], f16, tag="a2")
            asq2 = ap.tile([128, 128], f16, tag="asq2")
            for h in range(2):
                zp = z_half(tw1[:], tx[:], 8, h)
                tail_half(
                    zp, al1, be1, 2 * h,
                    a2[:, h * 64 : (h + 1) * 64],
                    asq2[:, h * 64 : (h + 1) * 64], 1, h,
                )

            # ---- layer 2 ----
            s2p = stats(a2[:], asq2[:], 4)
            al2, be2 = ab_from(s2p, "2")
            a3 = ap.tile([128, 128], f16, tag="a3")
            asq3 = ap.tile([128, 128], f16, tag="asq3")
            for h in range(2):
                zp = z_half(tw2[:], a2[:], 4, h)
                tail_half(
                    zp, al2, be2, 4 + 2 * h,
                    a3[:, h * 64 : (h + 1) * 64],
                    asq3[:, h * 64 : (h + 1) * 64], 2, h,
                )

            # ---- layer 3 (single 32-col output group per core) ----
            s3p = stats(a3[:], asq3[:], 4)
            al3, be3 = ab_from(s3p, "3")
            z3 = pp.tile([O3L, B], f32, tag="z3")
            for ic in range(4):
                nc.tensor.matmul(
                    z3[:],
                    tw3[:, ic * O3L : (ic + 1) * O3L],
                    a3[:, ic * B : (ic + 1) * B],
                    start=(ic == 0),
                    stop=(ic == 3),
                )
            vj3 = ap.tile([O3L, B], f16, tag="vj3")
            t13 = ap.tile([O3L, B], f32, tag="t13")
            out_sb = ap.tile([O3L, B], f32, tag="o3")
            nc.scalar.activation(
                vj3[:], z3[:], AF.Relu, bias=tmisc[0:O3L, 8:9], scale=1.0
            )
            nc.vector.scalar_tensor_tensor(
                t13[:], z3[:], k1, be3[0:O3L, 0:B], ALU.mult, ALU.add
            )
            nc.vector.tensor_tensor(vj3[:], vj3[:], al3[0:O3L, 0:B], ALU.mult)
            nc.vector.tensor_tensor(out_sb[:], vj3[:], t13[:], ALU.add)

            nc.sync.dma_start(out_d[:], out_sb[:])

    nc.compile()
    return nc


def kernel(**inputs):
    from concourse.bass_utils import run_bass_kernel_spmd

    x = np.asarray(inputs["x"], dtype=np.float32)
    fc1_w = np.asarray(inputs["fc1_w"], dtype=np.float32)
    fc1_b = np.asarray(inputs["fc1_b"], dtype=np.float32)
    fc2_w = np.asarray(inputs["fc2_w"], dtype=np.float32)
    fc2_b = np.asarray(inputs["fc2_b"], dtype=np.float32)
    fc3_w = np.asarray(inputs["fc3_w"], dtype=np.float32)
    fc3_b = np.asarray(inputs["fc3_b"], dtype=np.float32)
    c1w = np.asarray(inputs["conv1_w"], dtype=np.float32)
    c1b = np.asarray(inputs["conv1_b"], dtype=np.float32)
    c2w = np.asarray(inputs["conv2_w"], dtype=np.float32)
    c2b = np.asarray(inputs["conv2_b"], dtype=np.float32)
    bn = float(np.asarray(inputs["batch_num"]).astype(np.float64))

    scale = np.float32(RATE) / np.float32(bn)
    g = (c1w.T @ c2w[0]).astype(np.float32)  # [3]
    hb = np.float32(c1b @ c2w[0] + c2b[0])
    k0 = float(scale * g[0])
    k1 = float(scale * g[1])
    k2 = float(scale * g[2])
    kb = float(scale * hb)

    key = (k0, k1, k2, kb)
    if key not in _CACHE:
        _CACHE[key] = _build(*key)
    nc = _CACHE[key]

    def pack_mg(Wt, n_c, m_c, mw):
        # Wt [in, out] -> [128, m_c*n_c*mw], blocks (mg, ic) = Wt chunk
        out = np.empty((128, m_c * n_c * mw), dtype=np.float16)
        col = 0
        for mg in range(m_c):
            for ic in range(n_c):
                out[:, col : col + mw] = Wt[
                    ic * 128 : (ic + 1) * 128, mg * mw : (mg + 1) * mw
                ]
                col += mw
        return np.ascontiguousarray(out)

    w1_h = pack_mg(fc1_w.T, 8, 4, 128)
    w2_h = pack_mg(fc2_w.T, 4, 4, 128)
    xm_h = np.ascontiguousarray(
        x.T.reshape(8, 128, B).transpose(1, 0, 2).reshape(128, 8 * B)
    ).astype(np.float16)

    misc_h = np.zeros((128, 12), dtype=np.float32)
    misc_h[:, 0:4] = fc1_b.reshape(4, 128).T
    misc_h[:, 4:8] = fc2_b.reshape(4, 128).T

    in_maps = []
    for c in range(N_CORES):
        w3_h = pack_mg(fc3_w[c * O3L : (c + 1) * O3L].T, 4, 1, O3L)
        m_h = misc_h.copy()
        m_h[0:O3L, 8] = fc3_b[c * O3L : (c + 1) * O3L]
        in_maps.append(
            dict(xm=xm_h, misc=m_h, w1m=w1_h, w2m=w2_h, w3m=w3_h)
        )

    res = run_bass_kernel_spmd(nc, in_maps, list(range(N_CORES)))
    global LAST_RESULTS
    LAST_RESULTS = res
    return np.ascontiguousarray(
        np.concatenate([res.results[c]["out"].T for c in range(N_CORES)], axis=1)
    ).astype(np.float32)


if __name__ == "__main__":
    rng = np.random.default_rng(0)

    def lin(fo, fi):
        bound = 1.0 / np.sqrt(fi)
        return (
            rng.uniform(-bound, bound, (fo, fi)).astype(np.float32),
            rng.uniform(-bound, bound, (fo,)).astype(np.float32),
        )

    fc1_w, fc1_b = lin(512, 1024)
    fc2_w, fc2_b = lin(512, 512)
    fc3_w, fc3_b = lin(256, 512)
    c1w, c1b = lin(8, 3)
    c2w, c2b = lin(1, 8)
    ins = dict(
        x=rng.standard_normal((32, 1024)).astype(np.float32),
        fc1_w=fc1_w, fc1_b=fc1_b, fc2_w=fc2_w, fc2_b=fc2_b,
        fc3_w=fc3_w, fc3_b=fc3_b,
        conv1_w=c1w, conv1_b=c1b, conv2_w=c2w, conv2_b=c2b,
        batch_num=10,
    )
    out = kernel(**ins)
    print("kernel out", out.shape, out.dtype, float(np.abs(out).max()))
